# revision 8
# baseline (speedup 1.0000x reference)
"""Trainium2 Bass kernel for an AttentionBlock (GroupNorm -> QKV 1x1 ->
single-head attention over 1024 tokens -> out 1x1 -> residual).

Sharding: data-parallel over batch, 2 images per core on 8 NeuronCores.
All matmuls run in fp16 on the TensorEngine (f32 PSUM accumulate).

Layout strategy per image (x as [c=512, n=1024], c on partitions in 4 tiles):
  - GroupNorm stats: per 128-channel tile, sum(x) via DVE reduce and
    sum(x^2) via ACT Square's accum_out (two engines in parallel), then one
    128x128 block-diagonal mask matmul that segment-averages groups of 16
    partitions AND broadcasts the result back per channel in a single PE op.
  - qkv = Wqkv @ xn with W^T pre-transposed on host; q,k land as [c, n].
  - v is produced directly TRANSPOSED ([n, c]) by using xn as the stationary
    operand; scores are computed TRANSPOSED as well (ET[m, n], stationary k,
    moving q), so attn@v needs no transpose at all.
  - softmax without max subtraction (scores ~ N(0,1); f32 exp is safe).
    Row sums (over m = partitions) via a ones-vector matmul accumulated over
    the 8 m-tiles; 1/rowsum broadcast to all partitions by GpSimd and applied
    at the attn-output evacuation (normalization commutes through the final
    projection).
  - out = Wout @ attn_out with bias+residual fused into one DVE op.
"""

import numpy as np

import concourse.bass as bass
import concourse.tile as tile
from concourse import bacc, mybir
from concourse.bass_utils import run_bass_kernel_spmd

F32 = mybir.dt.float32
F16 = mybir.dt.float16

B, C, H, W = 16, 512, 32, 32
N = H * W            # 1024 tokens
G = 32               # groups
GS = C // G          # 16 channels per group
NCORES = 8
BL = B // NCORES     # 2 images per core
CT = C // 128        # 4 channel tiles
NB = N // 128        # 8 token blocks
EPS = 1e-5
SCALE = float(C) ** -0.5

_ALU = mybir.AluOpType
_ACT = mybir.ActivationFunctionType


def _build_body(ctx, tc, x_ext, wqkvT, woutT, bqk, bout, gns, gnb,
                gmask, out_ext):
    nc = tc.nc

    consts = ctx.enter_context(tc.tile_pool(name="consts", bufs=1))
    xpool = ctx.enter_context(tc.tile_pool(name="xp", bufs=2))
    h16 = ctx.enter_context(tc.tile_pool(name="h16", bufs=3))
    qpool = ctx.enter_context(tc.tile_pool(name="qp", bufs=2))
    kpool = ctx.enter_context(tc.tile_pool(name="kp", bufs=2))
    vpool = ctx.enter_context(tc.tile_pool(name="vp", bufs=2))
    big = ctx.enter_context(tc.tile_pool(name="bigp", bufs=3))
    rpool = ctx.enter_context(tc.tile_pool(name="rp", bufs=2))
    small = ctx.enter_context(tc.tile_pool(name="smallp", bufs=4))
    psum = ctx.enter_context(tc.tile_pool(name="psum", bufs=5, space="PSUM"))
    psum_s = ctx.enter_context(tc.tile_pool(name="psum_s", bufs=2, space="PSUM"))

    # ---- x of image 0 first: everything at startup waits on it ----
    x_tiles = [xpool.tile([128, CT, N], F32, tag="x", name=f"x_sb{i}")
               for i in range(BL)]
    for t in range(CT):
        nc.sync.dma_start(out=x_tiles[0][:, t, :], in_=x_ext[0, :, t, :])
    gmask_sb = consts.tile([128, 128], F32)
    nc.sync.dma_start(out=gmask_sb, in_=gmask)
    gns_sb = consts.tile([128, CT], F32)
    nc.sync.dma_start(out=gns_sb, in_=gns)
    gnb_sb = consts.tile([128, CT], F32)
    nc.sync.dma_start(out=gnb_sb, in_=gnb)
    eps_sb = consts.tile([128, 1], F32)
    nc.vector.memset(eps_sb, EPS)
    ones_sb = consts.tile([128, 128], F16)
    nc.vector.memset(ones_sb, 1.0)

    # ---- remaining constants ----
    wq_sb = consts.tile([128, CT, 3 * C], F16)
    nc.sync.dma_start(out=wq_sb, in_=wqkvT)
    wo_sb = consts.tile([128, CT, C], F16)
    nc.sync.dma_start(out=wo_sb, in_=woutT)
    bqk_sb = consts.tile([128, 2 * CT], F32)
    nc.sync.dma_start(out=bqk_sb, in_=bqk)
    bout_sb = consts.tile([128, CT], F32)
    nc.sync.dma_start(out=bout_sb, in_=bout)

    xn_tiles = []
    for img in range(BL):
        x_sb = x_tiles[img]
        if img > 0:
            for t in range(CT):
                nc.sync.dma_start(out=x_sb[:, t, :], in_=x_ext[img, :, t, :])

        # ---- group norm ----
        xn_sb = h16.tile([128, CT, N], F16, tag="h16", name=f"xn_sb{img}")
        xn_tiles.append(xn_sb)
        for t in range(CT):
            # tmp = (sum x, sum x^2) per partition; two engines in parallel.
            tmp = small.tile([128, 2], F32, tag="tmp")
            nc.vector.tensor_reduce(out=tmp[:, 0:1], in_=x_sb[:, t, :],
                                    axis=mybir.AxisListType.X, op=_ALU.add)
            nc.scalar.activation(out=xn_sb[:, t, :], in_=x_sb[:, t, :],
                                 func=_ACT.Square, accum_out=tmp[:, 1:2])
            # (mean_g, E[x^2]_g) broadcast per channel; gmask holds 1/16384
            gst = psum_s.tile([128, 2], F32, tag="gst", bufs=1)
            nc.tensor.matmul(gst, lhsT=gmask_sb, rhs=tmp, start=True, stop=True)
            # negv = mean^2 - E[x^2]  (= -var)
            negv = small.tile([128, 1], F32, tag="negv")
            nc.vector.tensor_scalar(
                out=negv, in0=gst[:, 0:1], scalar1=gst[:, 0:1],
                scalar2=gst[:, 1:2], op0=_ALU.mult, op1=_ALU.subtract)
            sqv = small.tile([128, 1], F32, tag="sqv")
            nc.scalar.activation(out=sqv, in_=negv, func=_ACT.Sqrt,
                                 bias=eps_sb, scale=-1.0)
            rstd = small.tile([128, 1], F32, tag="rstd")
            nc.vector.reciprocal(out=rstd, in_=sqv)
            a_sc = small.tile([128, 1], F32, tag="a_sc")
            nc.vector.tensor_mul(a_sc, rstd, gns_sb[:, t:t + 1])
            b_sc = small.tile([128, 1], F32, tag="b_sc")
            nc.vector.tensor_scalar(
                out=b_sc, in0=gst[:, 0:1], scalar1=a_sc,
                scalar2=gnb_sb[:, t:t + 1], op0=_ALU.mult, op1=_ALU.subtract)
            # xn = x*a - b  (fp16), overwrites the Square scratch
            nc.vector.tensor_scalar(
                out=xn_sb[:, t, :], in0=x_sb[:, t, :],
                scalar1=a_sc, scalar2=b_sc, op0=_ALU.mult, op1=_ALU.subtract)

    for img in range(BL):
        x_sb = x_tiles[img]
        xn_sb = xn_tiles[img]
        # ---- qkv projections ----
        q_sb = qpool.tile([128, CT, N], F16, tag="q")
        k_sb = kpool.tile([128, CT, N], F16, tag="k")
        for ob in range(2 * CT):
            dst = q_sb if ob < CT else k_sb
            od = ob % CT
            for h in range(2):
                pq = psum.tile([128, 512], F32, tag="mm")
                for ct in range(CT):
                    nc.tensor.matmul(
                        pq, lhsT=wq_sb[:, ct, ob * 128:(ob + 1) * 128],
                        rhs=xn_sb[:, ct, h * 512:(h + 1) * 512],
                        start=(ct == 0), stop=(ct == CT - 1))
                nc.scalar.activation(
                    out=dst[:, od, h * 512:(h + 1) * 512], in_=pq,
                    func=_ACT.Identity, bias=bqk_sb[:, ob:ob + 1], scale=1.0)
        # v, produced transposed: vT[m, c] (m on partitions)
        vT_sb = vpool.tile([128, NB, 512], F16, tag="v")
        for mb in range(NB):
            pv = psum.tile([128, 512], F32, tag="mm")
            for ct in range(CT):
                nc.tensor.matmul(
                    pv, lhsT=xn_sb[:, ct, mb * 128:(mb + 1) * 128],
                    rhs=wq_sb[:, ct, 2 * C:3 * C],
                    start=(ct == 0), stop=(ct == CT - 1))
            nc.vector.tensor_copy(out=vT_sb[:, mb, :], in_=pv)

        # ---- transposed scores ET[m, n] = exp(k.q/sqrt(c)), no max-sub ----
        et_sb = big.tile([128, NB, N], F16, tag="big")
        rs_ps = [psum_s.tile([128, 512], F32, tag="rs", bufs=2,
                             name=f"rs{img}_{i}") for i in range(2)]
        for mb in range(NB):
            for nh in range(2):
                ps = psum.tile([128, 512], F32, tag="mm")
                for ct in range(CT):
                    nc.tensor.matmul(
                        ps, lhsT=k_sb[:, ct, mb * 128:(mb + 1) * 128],
                        rhs=q_sb[:, ct, nh * 512:(nh + 1) * 512],
                        start=(ct == 0), stop=(ct == CT - 1))
                nc.scalar.activation(
                    out=et_sb[:, mb, nh * 512:(nh + 1) * 512], in_=ps,
                    func=_ACT.Exp, scale=SCALE)
                # row sums over m: accumulate ones^T @ ET across m-tiles
                nc.tensor.matmul(
                    rs_ps[nh], lhsT=ones_sb,
                    rhs=et_sb[:, mb, nh * 512:(nh + 1) * 512],
                    start=(mb == 0), stop=(mb == NB - 1))
        # 1/rowsum (already broadcast across partitions by the ones matmul)
        rrsb = rpool.tile([128, N], F32, tag="rrsb")
        for nh in range(2):
            nc.vector.reciprocal(out=rrsb[:, nh * 512:(nh + 1) * 512],
                                 in_=rs_ps[nh])

        # ---- attn @ v -> out_a^T [c, n], normalized + bias at evac ----
        oa_sb = h16.tile([128, CT, N], F16, tag="h16")
        for cb in range(CT):
            for nh in range(2):
                po = psum.tile([128, 512], F32, tag="mm")
                for mb in range(NB):
                    nc.tensor.matmul(
                        po, lhsT=vT_sb[:, mb, cb * 128:(cb + 1) * 128],
                        rhs=et_sb[:, mb, nh * 512:(nh + 1) * 512],
                        start=(mb == 0), stop=(mb == NB - 1))
                sl = slice(nh * 512, (nh + 1) * 512)
                nc.vector.tensor_mul(oa_sb[:, cb, sl], po, rrsb[:, sl])

        # ---- out projection + bias + residual ----
        y_sb = big.tile([128, CT, N], F32, tag="big")
        for ob in range(CT):
            for nh in range(2):
                pp = psum.tile([128, 512], F32, tag="mm")
                for cb in range(CT):
                    nc.tensor.matmul(
                        pp, lhsT=wo_sb[:, cb, ob * 128:(ob + 1) * 128],
                        rhs=oa_sb[:, cb, nh * 512:(nh + 1) * 512],
                        start=(cb == 0), stop=(cb == CT - 1))
                nc.vector.scalar_tensor_tensor(
                    out=y_sb[:, ob, nh * 512:(nh + 1) * 512], in0=pp,
                    scalar=bout_sb[:, ob:ob + 1],
                    in1=x_sb[:, ob, nh * 512:(nh + 1) * 512],
                    op0=_ALU.add, op1=_ALU.add)
            nc.sync.dma_start(out=out_ext[img, :, ob, :], in_=y_sb[:, ob, :])


def build():
    from contextlib import ExitStack

    nc = bacc.Bacc("TRN2", target_bir_lowering=False, debug=False,
                   num_devices=NCORES)
    x_ext = nc.dram_tensor("x", [BL, 128, CT, N], F32, kind="ExternalInput").ap()
    wqkvT = nc.dram_tensor("wqkvT", [128, CT, 3 * C], F16, kind="ExternalInput").ap()
    woutT = nc.dram_tensor("woutT", [128, CT, C], F16, kind="ExternalInput").ap()
    bqk = nc.dram_tensor("bqk", [128, 2 * CT], F32, kind="ExternalInput").ap()
    bout = nc.dram_tensor("bout", [128, CT], F32, kind="ExternalInput").ap()
    gns = nc.dram_tensor("gns", [128, CT], F32, kind="ExternalInput").ap()
    gnb = nc.dram_tensor("gnb", [128, CT], F32, kind="ExternalInput").ap()
    gmask = nc.dram_tensor("gmask", [128, 128], F32, kind="ExternalInput").ap()
    out_ext = nc.dram_tensor("out", [BL, 128, CT, N], F32, kind="ExternalOutput").ap()

    with tile.TileContext(nc) as tc:
        with ExitStack() as ctx:
            _build_body(ctx, tc, x_ext, wqkvT, woutT, bqk, bout, gns,
                        gnb, gmask, out_ext)
    nc.compile()
    return nc


_NC_CACHE = None


def _get_nc():
    global _NC_CACHE
    if _NC_CACHE is None:
        _NC_CACHE = build()
    return _NC_CACHE


def make_in_maps(inputs):
    x = np.asarray(inputs["x"], dtype=np.float32).reshape(B, C, N)
    w_qkv = np.asarray(inputs["w_qkv"], dtype=np.float32)
    b_qkv = np.asarray(inputs["b_qkv"], dtype=np.float32)
    w_out = np.asarray(inputs["w_out"], dtype=np.float32)
    b_out = np.asarray(inputs["b_out"], dtype=np.float32)
    gn_scale = np.asarray(inputs["gn_scale"], dtype=np.float32)
    gn_bias = np.asarray(inputs["gn_bias"], dtype=np.float32)

    def ptile(v):  # [t*128, ...] -> [128, t, ...]
        t = v.shape[0] // 128
        return np.ascontiguousarray(v.reshape(t, 128, *v.shape[1:]).transpose(
            1, 0, *range(2, v.ndim + 1)))

    wqkvT = ptile(w_qkv.T.astype(np.float16))
    woutT = ptile(w_out.T.astype(np.float16))
    bqk = ptile(b_qkv[:2 * C].reshape(2 * C, 1))[:, :, 0]
    bout_eff = (b_out.astype(np.float64)
                + w_out.astype(np.float64) @ b_qkv[2 * C:].astype(np.float64))
    bout = ptile(bout_eff.astype(np.float32).reshape(C, 1))[:, :, 0]
    gns = ptile(gn_scale.reshape(C, 1))[:, :, 0]
    gnb = ptile(gn_bias.reshape(C, 1))[:, :, 0]

    gmask = np.zeros((128, 128), dtype=np.float32)
    for g in range(128 // GS):
        gmask[g * GS:(g + 1) * GS, g * GS:(g + 1) * GS] = 1.0 / (GS * N)

    shared = {
        "wqkvT": wqkvT, "woutT": woutT, "bqk": bqk,
        "bout": bout, "gns": gns, "gnb": gnb, "gmask": gmask,
    }
    # x: [B, C, N] -> per core [BL, 128, CT, N]
    xp = np.ascontiguousarray(
        x.reshape(B, CT, 128, N).transpose(0, 2, 1, 3))
    in_maps = []
    for core in range(NCORES):
        m = dict(shared)
        m["x"] = np.ascontiguousarray(xp[core * BL:(core + 1) * BL])
        in_maps.append(m)
    return in_maps


def kernel(**inputs):
    nc = _get_nc()
    in_maps = make_in_maps(inputs)
    res = run_bass_kernel_spmd(nc, in_maps, core_ids=list(range(NCORES)))
    out = np.concatenate([res.results[i]["out"] for i in range(NCORES)], axis=0)
    # [B, 128, CT, N] -> [B, C, H, W]
    out = out.transpose(0, 2, 1, 3).reshape(B, C, H, W)
    return np.ascontiguousarray(out.astype(np.float32))


# revision 9
# speedup vs baseline: 1.1767x; 1.1767x over previous
"""Trainium2 Bass kernel for an AttentionBlock (GroupNorm -> QKV 1x1 ->
single-head attention over 1024 tokens -> out 1x1 -> residual).

Sharding: data-parallel over batch, 2 images per core on 8 NeuronCores.
All matmuls run in fp16 on the TensorEngine (f32 PSUM accumulate).

Layout strategy per image (x as [c=512, n=1024], c on partitions in 4 tiles):
  - GroupNorm stats: per 128-channel tile, sum(x) via DVE reduce and
    sum(x^2) via ACT Square's accum_out (two engines in parallel), then one
    128x128 block-diagonal mask matmul that segment-averages groups of 16
    partitions AND broadcasts the result back per channel in a single PE op.
  - qkv = Wqkv @ xn with W^T pre-transposed on host; q,k land as [c, n].
  - v is produced directly TRANSPOSED ([n, c]) by using xn as the stationary
    operand; scores are computed TRANSPOSED as well (ET[m, n], stationary k,
    moving q), so attn@v needs no transpose at all.
  - softmax without max subtraction (scores ~ N(0,1); f32 exp is safe).
    Row sums (over m = partitions) via a ones-vector matmul accumulated over
    the 8 m-tiles; 1/rowsum broadcast to all partitions by GpSimd and applied
    at the attn-output evacuation (normalization commutes through the final
    projection).
  - out = Wout @ attn_out with bias+residual fused into one DVE op.
"""

import numpy as np

import concourse.bass as bass
import concourse.tile as tile
from concourse import bacc, mybir
from concourse.bass_utils import run_bass_kernel_spmd

F32 = mybir.dt.float32
F16 = mybir.dt.float16

B, C, H, W = 16, 512, 32, 32
N = H * W            # 1024 tokens
G = 32               # groups
GS = C // G          # 16 channels per group
NCORES = 8
BL = B // NCORES     # 2 images per core
CT = C // 128        # 4 channel tiles
NB = N // 128        # 8 token blocks
EPS = 1e-5
SCALE = float(C) ** -0.5

_ALU = mybir.AluOpType
_ACT = mybir.ActivationFunctionType


def _build_body(ctx, tc, x_ext, wqkvT, woutT, bqk, bout, gns, gnb,
                gmask, out_ext):
    nc = tc.nc

    consts = ctx.enter_context(tc.tile_pool(name="consts", bufs=1))
    xpool = ctx.enter_context(tc.tile_pool(name="xp", bufs=2))
    h16 = ctx.enter_context(tc.tile_pool(name="h16", bufs=3))
    qpool = ctx.enter_context(tc.tile_pool(name="qp", bufs=2))
    kpool = ctx.enter_context(tc.tile_pool(name="kp", bufs=2))
    vpool = ctx.enter_context(tc.tile_pool(name="vp", bufs=2))
    big = ctx.enter_context(tc.tile_pool(name="bigp", bufs=3))
    rpool = ctx.enter_context(tc.tile_pool(name="rp", bufs=2))
    small = ctx.enter_context(tc.tile_pool(name="smallp", bufs=4))
    psum = ctx.enter_context(tc.tile_pool(name="psum", bufs=5, space="PSUM"))
    psum_s = ctx.enter_context(tc.tile_pool(name="psum_s", bufs=2, space="PSUM"))

    # ---- x of image 0 first: everything at startup waits on it ----
    x_tiles = [xpool.tile([128, CT, N], F32, tag="x", name=f"x_sb{i}")
               for i in range(BL)]
    for t in range(CT):
        nc.sync.dma_start(out=x_tiles[0][:, t, :], in_=x_ext[0, :, t, :])
    gmask_sb = consts.tile([128, 128], F32)
    nc.sync.dma_start(out=gmask_sb, in_=gmask)
    gns_sb = consts.tile([128, CT], F32)
    nc.sync.dma_start(out=gns_sb, in_=gns)
    gnb_sb = consts.tile([128, CT], F32)
    nc.sync.dma_start(out=gnb_sb, in_=gnb)
    eps_sb = consts.tile([128, 1], F32)
    nc.vector.memset(eps_sb, EPS)
    ones_sb = consts.tile([128, 128], F16)
    nc.vector.memset(ones_sb, 1.0)

    # ---- remaining constants ----
    wq_sb = consts.tile([128, CT, 3 * C], F16)
    nc.sync.dma_start(out=wq_sb, in_=wqkvT)
    wo_sb = consts.tile([128, CT, C], F16)
    nc.sync.dma_start(out=wo_sb, in_=woutT)
    bqk_sb = consts.tile([128, 2 * CT], F32)
    nc.sync.dma_start(out=bqk_sb, in_=bqk)
    bout_sb = consts.tile([128, CT], F32)
    nc.sync.dma_start(out=bout_sb, in_=bout)

    xn_tiles = []
    for img in range(BL):
        x_sb = x_tiles[img]
        if img > 0:
            for t in range(CT):
                nc.sync.dma_start(out=x_sb[:, t, :], in_=x_ext[img, :, t, :])

        # ---- group norm ----
        xn_sb = h16.tile([128, CT, N], F16, tag="h16", name=f"xn_sb{img}")
        xn_tiles.append(xn_sb)
        for t in range(CT):
            # tmp = (sum x, sum x^2) per partition; two engines in parallel.
            tmp = small.tile([128, 2], F32, tag="tmp")
            nc.vector.tensor_reduce(out=tmp[:, 0:1], in_=x_sb[:, t, :],
                                    axis=mybir.AxisListType.X, op=_ALU.add)
            nc.scalar.activation(out=xn_sb[:, t, :], in_=x_sb[:, t, :],
                                 func=_ACT.Square, accum_out=tmp[:, 1:2])
            # (mean_g, E[x^2]_g) broadcast per channel; gmask holds 1/16384
            gst = psum_s.tile([128, 2], F32, tag="gst", bufs=1)
            nc.tensor.matmul(gst, lhsT=gmask_sb, rhs=tmp, start=True, stop=True)
            # negv = mean^2 - E[x^2]  (= -var)
            negv = small.tile([128, 1], F32, tag="negv")
            nc.vector.tensor_scalar(
                out=negv, in0=gst[:, 0:1], scalar1=gst[:, 0:1],
                scalar2=gst[:, 1:2], op0=_ALU.mult, op1=_ALU.subtract)
            sqv = small.tile([128, 1], F32, tag="sqv")
            nc.scalar.activation(out=sqv, in_=negv, func=_ACT.Sqrt,
                                 bias=eps_sb, scale=-1.0)
            rstd = small.tile([128, 1], F32, tag="rstd")
            nc.vector.reciprocal(out=rstd, in_=sqv)
            a_sc = small.tile([128, 1], F32, tag="a_sc")
            nc.vector.tensor_mul(a_sc, rstd, gns_sb[:, t:t + 1])
            b_sc = small.tile([128, 1], F32, tag="b_sc")
            nc.vector.tensor_scalar(
                out=b_sc, in0=gst[:, 0:1], scalar1=a_sc,
                scalar2=gnb_sb[:, t:t + 1], op0=_ALU.mult, op1=_ALU.subtract)
            # xn = x*a - b  (fp16), overwrites the Square scratch
            nc.vector.tensor_scalar(
                out=xn_sb[:, t, :], in0=x_sb[:, t, :],
                scalar1=a_sc, scalar2=b_sc, op0=_ALU.mult, op1=_ALU.subtract)

    for img in range(BL):
        x_sb = x_tiles[img]
        xn_sb = xn_tiles[img]
        # ---- qkv projections ----
        q_sb = qpool.tile([128, CT, N], F16, tag="q")
        k_sb = kpool.tile([128, CT, N], F16, tag="k")
        for ob in range(2 * CT):
            dst = q_sb if ob < CT else k_sb
            od = ob % CT
            for h in range(2):
                pq = psum.tile([128, 512], F32, tag="mm")
                for ct in range(CT):
                    nc.tensor.matmul(
                        pq, lhsT=wq_sb[:, ct, ob * 128:(ob + 1) * 128],
                        rhs=xn_sb[:, ct, h * 512:(h + 1) * 512],
                        start=(ct == 0), stop=(ct == CT - 1))
                nc.scalar.activation(
                    out=dst[:, od, h * 512:(h + 1) * 512], in_=pq,
                    func=_ACT.Identity, bias=bqk_sb[:, ob:ob + 1], scale=1.0)
        # v, produced transposed: vT[m, c] (m on partitions)
        vT_sb = vpool.tile([128, NB, 512], F16, tag="v")
        for mb in range(NB):
            pv = psum.tile([128, 512], F32, tag="mm")
            for ct in range(CT):
                nc.tensor.matmul(
                    pv, lhsT=xn_sb[:, ct, mb * 128:(mb + 1) * 128],
                    rhs=wq_sb[:, ct, 2 * C:3 * C],
                    start=(ct == 0), stop=(ct == CT - 1))
            nc.vector.tensor_copy(out=vT_sb[:, mb, :], in_=pv)

        # ---- transposed scores ET[m, n] = exp(k.q/sqrt(c)), no max-sub ----
        et_sb = big.tile([128, NB, N], F16, tag="big")
        rs_ps = [psum_s.tile([128, 512], F32, tag="rs", bufs=2,
                             name=f"rs{img}_{i}") for i in range(2)]
        for mb in range(NB):
            for nh in range(2):
                ps = psum.tile([128, 512], F32, tag="mm")
                for ct in range(CT):
                    nc.tensor.matmul(
                        ps, lhsT=k_sb[:, ct, mb * 128:(mb + 1) * 128],
                        rhs=q_sb[:, ct, nh * 512:(nh + 1) * 512],
                        start=(ct == 0), stop=(ct == CT - 1))
                nc.scalar.activation(
                    out=et_sb[:, mb, nh * 512:(nh + 1) * 512], in_=ps,
                    func=_ACT.Exp, scale=SCALE)
        # row sums over m via ones matmuls, batched after the scores phase so
        # they do not break the scores matmuls' weight-load pipelining; the
        # all-ones stationary broadcasts the sum to every partition.
        for nh in range(2):
            for mb in range(NB):
                nc.tensor.matmul(
                    rs_ps[nh], lhsT=ones_sb,
                    rhs=et_sb[:, mb, nh * 512:(nh + 1) * 512],
                    start=(mb == 0), stop=(mb == NB - 1))
        rrsb = rpool.tile([128, N], F32, tag="rrsb")
        for nh in range(2):
            nc.vector.reciprocal_approx_fast(
                out=rrsb[:, nh * 512:(nh + 1) * 512], in_=rs_ps[nh])

        # ---- attn @ v -> out_a^T [c, n], normalized + bias at evac ----
        oa_sb = h16.tile([128, CT, N], F16, tag="h16")
        for cb in range(CT):
            for nh in range(2):
                po = psum.tile([128, 512], F32, tag="mm")
                for mb in range(NB):
                    nc.tensor.matmul(
                        po, lhsT=vT_sb[:, mb, cb * 128:(cb + 1) * 128],
                        rhs=et_sb[:, mb, nh * 512:(nh + 1) * 512],
                        start=(mb == 0), stop=(mb == NB - 1))
                sl = slice(nh * 512, (nh + 1) * 512)
                nc.vector.tensor_mul(oa_sb[:, cb, sl], po, rrsb[:, sl])

        # ---- out projection + bias + residual ----
        y_sb = big.tile([128, CT, N], F32, tag="big")
        for ob in range(CT):
            for nh in range(2):
                pp = psum.tile([128, 512], F32, tag="mm")
                for cb in range(CT):
                    nc.tensor.matmul(
                        pp, lhsT=wo_sb[:, cb, ob * 128:(ob + 1) * 128],
                        rhs=oa_sb[:, cb, nh * 512:(nh + 1) * 512],
                        start=(cb == 0), stop=(cb == CT - 1))
                nc.vector.scalar_tensor_tensor(
                    out=y_sb[:, ob, nh * 512:(nh + 1) * 512], in0=pp,
                    scalar=bout_sb[:, ob:ob + 1],
                    in1=x_sb[:, ob, nh * 512:(nh + 1) * 512],
                    op0=_ALU.add, op1=_ALU.add)
            nc.sync.dma_start(out=out_ext[img, :, ob, :], in_=y_sb[:, ob, :])


def build():
    from contextlib import ExitStack

    nc = bacc.Bacc("TRN2", target_bir_lowering=False, debug=False,
                   num_devices=NCORES)
    x_ext = nc.dram_tensor("x", [BL, 128, CT, N], F32, kind="ExternalInput").ap()
    wqkvT = nc.dram_tensor("wqkvT", [128, CT, 3 * C], F16, kind="ExternalInput").ap()
    woutT = nc.dram_tensor("woutT", [128, CT, C], F16, kind="ExternalInput").ap()
    bqk = nc.dram_tensor("bqk", [128, 2 * CT], F32, kind="ExternalInput").ap()
    bout = nc.dram_tensor("bout", [128, CT], F32, kind="ExternalInput").ap()
    gns = nc.dram_tensor("gns", [128, CT], F32, kind="ExternalInput").ap()
    gnb = nc.dram_tensor("gnb", [128, CT], F32, kind="ExternalInput").ap()
    gmask = nc.dram_tensor("gmask", [128, 128], F32, kind="ExternalInput").ap()
    out_ext = nc.dram_tensor("out", [BL, 128, CT, N], F32, kind="ExternalOutput").ap()

    with tile.TileContext(nc) as tc:
        with ExitStack() as ctx:
            _build_body(ctx, tc, x_ext, wqkvT, woutT, bqk, bout, gns,
                        gnb, gmask, out_ext)
    nc.compile()
    return nc


_NC_CACHE = None


def _get_nc():
    global _NC_CACHE
    if _NC_CACHE is None:
        _NC_CACHE = build()
    return _NC_CACHE


def make_in_maps(inputs):
    x = np.asarray(inputs["x"], dtype=np.float32).reshape(B, C, N)
    w_qkv = np.asarray(inputs["w_qkv"], dtype=np.float32)
    b_qkv = np.asarray(inputs["b_qkv"], dtype=np.float32)
    w_out = np.asarray(inputs["w_out"], dtype=np.float32)
    b_out = np.asarray(inputs["b_out"], dtype=np.float32)
    gn_scale = np.asarray(inputs["gn_scale"], dtype=np.float32)
    gn_bias = np.asarray(inputs["gn_bias"], dtype=np.float32)

    def ptile(v):  # [t*128, ...] -> [128, t, ...]
        t = v.shape[0] // 128
        return np.ascontiguousarray(v.reshape(t, 128, *v.shape[1:]).transpose(
            1, 0, *range(2, v.ndim + 1)))

    wqkvT = ptile(w_qkv.T.astype(np.float16))
    woutT = ptile(w_out.T.astype(np.float16))
    bqk = ptile(b_qkv[:2 * C].reshape(2 * C, 1))[:, :, 0]
    bout_eff = (b_out.astype(np.float64)
                + w_out.astype(np.float64) @ b_qkv[2 * C:].astype(np.float64))
    bout = ptile(bout_eff.astype(np.float32).reshape(C, 1))[:, :, 0]
    gns = ptile(gn_scale.reshape(C, 1))[:, :, 0]
    gnb = ptile(gn_bias.reshape(C, 1))[:, :, 0]

    gmask = np.zeros((128, 128), dtype=np.float32)
    for g in range(128 // GS):
        gmask[g * GS:(g + 1) * GS, g * GS:(g + 1) * GS] = 1.0 / (GS * N)

    shared = {
        "wqkvT": wqkvT, "woutT": woutT, "bqk": bqk,
        "bout": bout, "gns": gns, "gnb": gnb, "gmask": gmask,
    }
    # x: [B, C, N] -> per core [BL, 128, CT, N]
    xp = np.ascontiguousarray(
        x.reshape(B, CT, 128, N).transpose(0, 2, 1, 3))
    in_maps = []
    for core in range(NCORES):
        m = dict(shared)
        m["x"] = np.ascontiguousarray(xp[core * BL:(core + 1) * BL])
        in_maps.append(m)
    return in_maps


def kernel(**inputs):
    nc = _get_nc()
    in_maps = make_in_maps(inputs)
    res = run_bass_kernel_spmd(nc, in_maps, core_ids=list(range(NCORES)))
    out = np.concatenate([res.results[i]["out"] for i in range(NCORES)], axis=0)
    # [B, 128, CT, N] -> [B, C, H, W]
    out = out.transpose(0, 2, 1, 3).reshape(B, C, H, W)
    return np.ascontiguousarray(out.astype(np.float32))


# revision 11
# speedup vs baseline: 1.2499x; 1.0622x over previous
"""Trainium2 Bass kernel for an AttentionBlock (GroupNorm -> QKV 1x1 ->
single-head attention over 1024 tokens -> out 1x1 -> residual).

Sharding: data-parallel over batch, 2 images per core on 8 NeuronCores.
All matmuls run in fp16 on the TensorEngine (f32 PSUM accumulate).

Layout strategy per image (x as [c=512, n=1024], c on partitions in 4 tiles):
  - GroupNorm stats: per 128-channel tile, sum(x) via DVE reduce and
    sum(x^2) via ACT Square's accum_out (two engines in parallel), then one
    128x128 block-diagonal mask matmul that segment-averages groups of 16
    partitions AND broadcasts the result back per channel in a single PE op.
  - qkv = Wqkv @ xn with W^T pre-transposed on host; q,k land as [c, n].
  - v is produced directly TRANSPOSED ([n, c]) by using xn as the stationary
    operand; scores are computed TRANSPOSED as well (ET[m, n], stationary k,
    moving q), so attn@v needs no transpose at all.
  - softmax without max subtraction (scores ~ N(0,1); f32 exp is safe).
    Row sums (over m = partitions) via a ones-vector matmul accumulated over
    the 8 m-tiles; 1/rowsum broadcast to all partitions by GpSimd and applied
    at the attn-output evacuation (normalization commutes through the final
    projection).
  - out = Wout @ attn_out with bias+residual fused into one DVE op.
"""

import numpy as np

import concourse.bass as bass
import concourse.tile as tile
from concourse import bacc, mybir
from concourse.bass_utils import run_bass_kernel_spmd

F32 = mybir.dt.float32
F16 = mybir.dt.float16

B, C, H, W = 16, 512, 32, 32
N = H * W            # 1024 tokens
G = 32               # groups
GS = C // G          # 16 channels per group
NCORES = 8
BL = B // NCORES     # 2 images per core
CT = C // 128        # 4 channel tiles
NB = N // 128        # 8 token blocks
EPS = 1e-5
SCALE = float(C) ** -0.5

_ALU = mybir.AluOpType
_ACT = mybir.ActivationFunctionType


def _build_body(ctx, tc, x_ext, wqkvT, woutT, bqk, bout, gns, gnb,
                gmask, out_ext):
    nc = tc.nc

    consts = ctx.enter_context(tc.tile_pool(name="consts", bufs=1))
    xpool = ctx.enter_context(tc.tile_pool(name="xp", bufs=2))
    h16 = ctx.enter_context(tc.tile_pool(name="h16", bufs=3))
    qpool = ctx.enter_context(tc.tile_pool(name="qp", bufs=2))
    kpool = ctx.enter_context(tc.tile_pool(name="kp", bufs=2))
    vpool = ctx.enter_context(tc.tile_pool(name="vp", bufs=2))
    big = ctx.enter_context(tc.tile_pool(name="bigp", bufs=3))
    rpool = ctx.enter_context(tc.tile_pool(name="rp", bufs=2))
    small = ctx.enter_context(tc.tile_pool(name="smallp", bufs=4))
    psum = ctx.enter_context(tc.tile_pool(name="psum", bufs=6, space="PSUM"))
    psum_s = ctx.enter_context(tc.tile_pool(name="psum_s", bufs=2, space="PSUM"))

    # ---- x of image 0 first: everything at startup waits on it ----
    x_tiles = [xpool.tile([128, CT, N], F16, tag="x", name=f"x_sb{i}")
               for i in range(BL)]
    for t in range(CT):
        nc.sync.dma_start(out=x_tiles[0][:, t, :], in_=x_ext[0, :, t, :])
    gmask_sb = consts.tile([128, 128], F32)
    nc.sync.dma_start(out=gmask_sb, in_=gmask)
    gns_sb = consts.tile([128, CT], F32)
    nc.sync.dma_start(out=gns_sb, in_=gns)
    gnb_sb = consts.tile([128, CT], F32)
    nc.sync.dma_start(out=gnb_sb, in_=gnb)
    eps_sb = consts.tile([128, 1], F32)
    nc.vector.memset(eps_sb, EPS)
    ones_sb = consts.tile([128, 128], F16)
    nc.vector.memset(ones_sb, 1.0)

    # ---- remaining constants ----
    wq_sb = consts.tile([128, CT, 3 * C], F16)
    nc.sync.dma_start(out=wq_sb, in_=wqkvT)
    wo_sb = consts.tile([128, CT, C], F16)
    nc.sync.dma_start(out=wo_sb, in_=woutT)
    bqk_sb = consts.tile([128, 2 * CT], F32)
    nc.sync.dma_start(out=bqk_sb, in_=bqk)
    bout_sb = consts.tile([128, CT], F32)
    nc.sync.dma_start(out=bout_sb, in_=bout)

    xn_tiles = []
    for img in range(BL):
        x_sb = x_tiles[img]
        if img > 0:
            for t in range(CT):
                nc.sync.dma_start(out=x_sb[:, t, :], in_=x_ext[img, :, t, :])

        # ---- group norm ----
        xn_sb = h16.tile([128, CT, N], F16, tag="h16", name=f"xn_sb{img}")
        xn_tiles.append(xn_sb)
        for t in range(CT):
            # tmp = (sum x, sum x^2) per partition; two engines in parallel.
            tmp = small.tile([128, 2], F32, tag="tmp")
            nc.vector.tensor_reduce(out=tmp[:, 0:1], in_=x_sb[:, t, :],
                                    axis=mybir.AxisListType.X, op=_ALU.add)
            nc.scalar.activation(out=xn_sb[:, t, :], in_=x_sb[:, t, :],
                                 func=_ACT.Square, accum_out=tmp[:, 1:2])
            # (mean_g, E[x^2]_g) broadcast per channel; gmask holds 1/16384
            gst = psum_s.tile([128, 2], F32, tag="aux", bufs=2)
            nc.tensor.matmul(gst, lhsT=gmask_sb, rhs=tmp, start=True, stop=True)
            # negv = mean^2 - E[x^2]  (= -var)
            negv = small.tile([128, 1], F32, tag="negv")
            nc.vector.tensor_scalar(
                out=negv, in0=gst[:, 0:1], scalar1=gst[:, 0:1],
                scalar2=gst[:, 1:2], op0=_ALU.mult, op1=_ALU.subtract)
            sqv = small.tile([128, 1], F32, tag="sqv")
            nc.scalar.activation(out=sqv, in_=negv, func=_ACT.Sqrt,
                                 bias=eps_sb, scale=-1.0)
            rstd = small.tile([128, 1], F32, tag="rstd")
            nc.vector.reciprocal(out=rstd, in_=sqv)
            a_sc = small.tile([128, 1], F32, tag="a_sc")
            nc.vector.tensor_mul(a_sc, rstd, gns_sb[:, t:t + 1])
            b_sc = small.tile([128, 1], F32, tag="b_sc")
            nc.vector.tensor_scalar(
                out=b_sc, in0=gst[:, 0:1], scalar1=a_sc,
                scalar2=gnb_sb[:, t:t + 1], op0=_ALU.mult, op1=_ALU.subtract)
            # xn = x*a - b  (fp16), overwrites the Square scratch
            nc.vector.tensor_scalar(
                out=xn_sb[:, t, :], in0=x_sb[:, t, :],
                scalar1=a_sc, scalar2=b_sc, op0=_ALU.mult, op1=_ALU.subtract)

    for img in range(BL):
        x_sb = x_tiles[img]
        xn_sb = xn_tiles[img]
        # ---- qkv projections ----
        q_sb = qpool.tile([128, CT, N], F16, tag="q")
        k_sb = kpool.tile([128, CT, N], F16, tag="k")
        for ob in range(2 * CT):
            dst = q_sb if ob < CT else k_sb
            od = ob % CT
            for h in range(2):
                pq = psum.tile([128, 512], F32, tag="mm")
                for ct in range(CT):
                    nc.tensor.matmul(
                        pq, lhsT=wq_sb[:, ct, ob * 128:(ob + 1) * 128],
                        rhs=xn_sb[:, ct, h * 512:(h + 1) * 512],
                        start=(ct == 0), stop=(ct == CT - 1))
                nc.scalar.activation(
                    out=dst[:, od, h * 512:(h + 1) * 512], in_=pq,
                    func=_ACT.Identity, bias=bqk_sb[:, ob:ob + 1], scale=1.0)
        # v, produced transposed: vT[m, c] (m on partitions)
        vT_sb = vpool.tile([128, NB, 512], F16, tag="v")
        for mb in range(NB):
            pv = psum.tile([128, 512], F32, tag="mm")
            for ct in range(CT):
                nc.tensor.matmul(
                    pv, lhsT=xn_sb[:, ct, mb * 128:(mb + 1) * 128],
                    rhs=wq_sb[:, ct, 2 * C:3 * C],
                    start=(ct == 0), stop=(ct == CT - 1))
            nc.vector.tensor_copy(out=vT_sb[:, mb, :], in_=pv)

        # ---- transposed scores ET[m, n] = exp(k.q/sqrt(c)), no max-sub ----
        et_sb = big.tile([128, NB, N], F16, tag="big")
        rs_ps = [psum_s.tile([128, 512], F32, tag="aux", bufs=2,
                             name=f"rs{img}_{i}") for i in range(2)]
        for mb in range(NB):
            for nh in range(2):
                ps = psum.tile([128, 512], F32, tag="mm")
                for ct in range(CT):
                    nc.tensor.matmul(
                        ps, lhsT=k_sb[:, ct, mb * 128:(mb + 1) * 128],
                        rhs=q_sb[:, ct, nh * 512:(nh + 1) * 512],
                        start=(ct == 0), stop=(ct == CT - 1))
                nc.scalar.activation(
                    out=et_sb[:, mb, nh * 512:(nh + 1) * 512], in_=ps,
                    func=_ACT.Exp, scale=SCALE)
        # row sums over m via ones matmuls, batched after the scores phase so
        # they do not break the scores matmuls' weight-load pipelining; the
        # all-ones stationary broadcasts the sum to every partition.
        for nh in range(2):
            for mb in range(NB):
                nc.tensor.matmul(
                    rs_ps[nh], lhsT=ones_sb,
                    rhs=et_sb[:, mb, nh * 512:(nh + 1) * 512],
                    start=(mb == 0), stop=(mb == NB - 1))
        rrsb = rpool.tile([128, N], F32, tag="rrsb")
        for nh in range(2):
            nc.vector.reciprocal_approx_fast(
                out=rrsb[:, nh * 512:(nh + 1) * 512], in_=rs_ps[nh])

        # ---- attn @ v -> out_a^T [c, n], normalized + bias at evac ----
        oa_sb = h16.tile([128, CT, N], F16, tag="h16")
        for cb in range(CT):
            for nh in range(2):
                po = psum.tile([128, 512], F32, tag="mm")
                for mb in range(NB):
                    nc.tensor.matmul(
                        po, lhsT=vT_sb[:, mb, cb * 128:(cb + 1) * 128],
                        rhs=et_sb[:, mb, nh * 512:(nh + 1) * 512],
                        start=(mb == 0), stop=(mb == NB - 1))
                sl = slice(nh * 512, (nh + 1) * 512)
                nc.vector.tensor_mul(oa_sb[:, cb, sl], po, rrsb[:, sl])

        # ---- out projection + bias + residual ----
        y_sb = big.tile([128, CT, N], F16, tag="big")
        for ob in range(CT):
            for nh in range(2):
                pp = psum.tile([128, 512], F32, tag="mm")
                for cb in range(CT):
                    nc.tensor.matmul(
                        pp, lhsT=wo_sb[:, cb, ob * 128:(ob + 1) * 128],
                        rhs=oa_sb[:, cb, nh * 512:(nh + 1) * 512],
                        start=(cb == 0), stop=(cb == CT - 1))
                nc.vector.scalar_tensor_tensor(
                    out=y_sb[:, ob, nh * 512:(nh + 1) * 512], in0=pp,
                    scalar=bout_sb[:, ob:ob + 1],
                    in1=x_sb[:, ob, nh * 512:(nh + 1) * 512],
                    op0=_ALU.add, op1=_ALU.add)
            nc.sync.dma_start(out=out_ext[img, :, ob, :], in_=y_sb[:, ob, :])


def build():
    from contextlib import ExitStack

    nc = bacc.Bacc("TRN2", target_bir_lowering=False, debug=False,
                   num_devices=NCORES)
    x_ext = nc.dram_tensor("x", [BL, 128, CT, N], F16, kind="ExternalInput").ap()
    wqkvT = nc.dram_tensor("wqkvT", [128, CT, 3 * C], F16, kind="ExternalInput").ap()
    woutT = nc.dram_tensor("woutT", [128, CT, C], F16, kind="ExternalInput").ap()
    bqk = nc.dram_tensor("bqk", [128, 2 * CT], F32, kind="ExternalInput").ap()
    bout = nc.dram_tensor("bout", [128, CT], F32, kind="ExternalInput").ap()
    gns = nc.dram_tensor("gns", [128, CT], F32, kind="ExternalInput").ap()
    gnb = nc.dram_tensor("gnb", [128, CT], F32, kind="ExternalInput").ap()
    gmask = nc.dram_tensor("gmask", [128, 128], F32, kind="ExternalInput").ap()
    out_ext = nc.dram_tensor("out", [BL, 128, CT, N], F16, kind="ExternalOutput").ap()

    with tile.TileContext(nc) as tc:
        with ExitStack() as ctx:
            _build_body(ctx, tc, x_ext, wqkvT, woutT, bqk, bout, gns,
                        gnb, gmask, out_ext)
    nc.compile()
    return nc


_NC_CACHE = None


def _get_nc():
    global _NC_CACHE
    if _NC_CACHE is None:
        _NC_CACHE = build()
    return _NC_CACHE


def make_in_maps(inputs):
    x = np.asarray(inputs["x"], dtype=np.float32).reshape(B, C, N)
    w_qkv = np.asarray(inputs["w_qkv"], dtype=np.float32)
    b_qkv = np.asarray(inputs["b_qkv"], dtype=np.float32)
    w_out = np.asarray(inputs["w_out"], dtype=np.float32)
    b_out = np.asarray(inputs["b_out"], dtype=np.float32)
    gn_scale = np.asarray(inputs["gn_scale"], dtype=np.float32)
    gn_bias = np.asarray(inputs["gn_bias"], dtype=np.float32)

    def ptile(v):  # [t*128, ...] -> [128, t, ...]
        t = v.shape[0] // 128
        return np.ascontiguousarray(v.reshape(t, 128, *v.shape[1:]).transpose(
            1, 0, *range(2, v.ndim + 1)))

    wqkvT = ptile(w_qkv.T.astype(np.float16))
    woutT = ptile(w_out.T.astype(np.float16))
    bqk = ptile(b_qkv[:2 * C].reshape(2 * C, 1))[:, :, 0]
    bout_eff = (b_out.astype(np.float64)
                + w_out.astype(np.float64) @ b_qkv[2 * C:].astype(np.float64))
    bout = ptile(bout_eff.astype(np.float32).reshape(C, 1))[:, :, 0]
    gns = ptile(gn_scale.reshape(C, 1))[:, :, 0]
    gnb = ptile(gn_bias.reshape(C, 1))[:, :, 0]

    gmask = np.zeros((128, 128), dtype=np.float32)
    for g in range(128 // GS):
        gmask[g * GS:(g + 1) * GS, g * GS:(g + 1) * GS] = 1.0 / (GS * N)

    shared = {
        "wqkvT": wqkvT, "woutT": woutT, "bqk": bqk,
        "bout": bout, "gns": gns, "gnb": gnb, "gmask": gmask,
    }
    # x: [B, C, N] -> per core [BL, 128, CT, N]
    xp = np.ascontiguousarray(
        x.reshape(B, CT, 128, N).transpose(0, 2, 1, 3)).astype(np.float16)
    in_maps = []
    for core in range(NCORES):
        m = dict(shared)
        m["x"] = np.ascontiguousarray(xp[core * BL:(core + 1) * BL])
        in_maps.append(m)
    return in_maps


def kernel(**inputs):
    nc = _get_nc()
    in_maps = make_in_maps(inputs)
    res = run_bass_kernel_spmd(nc, in_maps, core_ids=list(range(NCORES)))
    out = np.concatenate([res.results[i]["out"] for i in range(NCORES)], axis=0)
    # [B, 128, CT, N] -> [B, C, H, W]
    out = out.transpose(0, 2, 1, 3).reshape(B, C, H, W)
    return np.ascontiguousarray(out.astype(np.float32))
